# revision 6
# baseline (speedup 1.0000x reference)
"""Trainium2 Bass kernel for nn_MultiHeadSelfAttention_88725434400988.

Self-contained: accepts FULL inputs, shards batch B=256 over 8 NeuronCores
(32 per core), runs one SPMD Bass program, gathers the FULL output.

Per-core schedule (B_CORE=32, S=8, F=32, E=64, A=64, NH=2), v3:
  - fp16 weights/activations, fp32 PSUM accumulation.
  - Head: hsT streams on the sync queue, hsv (2 halves) + act-table preload
    on scalar, tiny wv/wres/bias on gpsimd.  PE runs a short clock-warm
    burst then the FIRST HALF of the v projection (covers the weight-chunk
    DMA head), so real projection work starts ~8us in.
  - QK projection INTERLEAVED: chunk tg of Wq (sync queue) then chunk tg
    of Wk (scalar queue), alternating on the PE.  Both queues stream
    continuously at ~150GB/s each instead of two serial ~300GB/s phases,
    which removes the mid-kernel handoff stall and the LDWEIGHTS arrival
    gaps.  Each chunk is one DMA with 8KB-contiguous descriptors.
    Psum evacuation: rows 0:64 on ScalarE (Copy act), rows 64:128 on
    VectorE, halving per-group evac latency.
  - Partition shifts (psum rows 64:127 -> partitions 0:63 of qt/kt jh=1)
    ride the gpsimd/sync queues after their source stage fills; the kt
    shift is covered by the SECOND HALF of the v projection.
  - Attention: transpose-free, software-pipelined by 1 batch (see v1
    docstring): Z^T = k.T @ q per (nh, half), one big exp per batch on
    ScalarE -> bf16, denominators replicated via PE ones-matmul,
    reciprocal_approx_fast, residual per batch pair with fused bias+relu,
    fp16 output DMAs on the (idle by then) sync queue.
"""
import numpy as np

B, S, F, E, A, NH = 256, 8, 32, 64, 64, 2
NCORES = 8
BC = B // NCORES            # 32 batches per core
ROWS = BC * S               # 256 projection rows
CD = F * E                  # 2048 contraction dim
ND = A * F * NH             # 4096 projection cols
KTILES = CD // 128          # 16
TTILES = ND // 128          # 32 column tiles per weight
NB = BC * NH                # 64 attention batches per core
WCHUNK = 2                  # weight tiles per DMA chunk / psum group
NCHUNK = TTILES // WCHUNK   # 16 chunks per weight

_NC_CACHE = None


def build_bass():
    import concourse.bacc as bacc
    import concourse.tile as tile
    from concourse import mybir

    f16 = mybir.dt.float16
    bf16 = mybir.dt.bfloat16
    f32 = mybir.dt.float32
    Exp = mybir.ActivationFunctionType.Exp
    Copy = mybir.ActivationFunctionType.Copy
    Add = mybir.AluOpType.add
    Max = mybir.AluOpType.max

    nc = bacc.Bacc("TRN2", target_bir_lowering=False, debug=False)

    hst_d = nc.dram_tensor("hst", [128, KTILES, ROWS], f16, kind="ExternalInput")
    hsv_d = nc.dram_tensor("hsv", [E, NB, 128], f16, kind="ExternalInput")
    wq_d = nc.dram_tensor("wq", [128, TTILES, KTILES * 128], f16,
                          kind="ExternalInput")
    wk_d = nc.dram_tensor("wk", [128, TTILES, KTILES * 128], f16,
                          kind="ExternalInput")
    wv_d = nc.dram_tensor("wv", [E, 2 * A], f16, kind="ExternalInput")
    wres_d = nc.dram_tensor("wres", [2 * A, E], f16, kind="ExternalInput")
    bias_d = nc.dram_tensor("bias", [E, 1], f32, kind="ExternalInput")
    out_d = nc.dram_tensor("out", [128, (BC // 2) * 256], f16,
                           kind="ExternalOutput")

    with tile.TileContext(nc) as tc:
        from contextlib import ExitStack
        with ExitStack() as ctx:
            singles = ctx.enter_context(tc.tile_pool(name="singles", bufs=1))

            # ---- constants / persistent tiles ----
            ones_bf = singles.tile([128, A], bf16)
            nc.vector.memset(ones_bf, 1.0)
            warm_t = singles.tile([128, 256], f16)
            nc.vector.memset(warm_t, 0.25)
            dummy_e = singles.tile([128, 8], bf16)

            hsT = singles.tile([128, KTILES, ROWS], f16)
            hsv = singles.tile([E, NB, 128], f16)
            wv_sb = singles.tile([E, 2 * A], f16)
            wres_sb = singles.tile([128, 2, E], f16)
            bias_sb = singles.tile([128, 1], f32)

            qt = singles.tile([64, 2, BC, NH, 128], f16)
            kt_ = singles.tile([64, 2, BC, NH, 128], f16)
            # v with a ones column appended: av matmul col 64 = denominator
            v_all = singles.tile([128, NB, 2, A + 1], bf16)
            nc.vector.memset(v_all[:, :, :, A:A + 1], 1.0)
            ut = singles.tile([128, BC, 2, 128], f16)  # (nh,a) x (b, jh, f*4+sp)
            ident = singles.tile([128, 128], f16)
            from concourse.masks import make_identity
            make_identity(nc, ident[:, :])

            # ---- input DMAs: one bulk stream head per queue ----
            nc.sync.dma_start(hsT[:, :, :], hst_d[:])
            nc.scalar.dma_start(hsv[:, 0:NB // 2, :], hsv_d[:, 0:NB // 2, :])
            nc.scalar.dma_start(hsv[:, NB // 2:NB, :], hsv_d[:, NB // 2:NB, :])
            nc.gpsimd.dma_start(wv_sb[:, :], wv_d[:])
            for half in range(2):
                for jh in range(2):
                    nc.gpsimd.dma_start(
                        wres_sb[half * 64:(half + 1) * 64, jh, :],
                        wres_d[jh * 64:(jh + 1) * 64, :])
            nc.gpsimd.dma_start(bias_sb[0:64, :], bias_d[:])
            nc.gpsimd.dma_start(bias_sb[64:128, :], bias_d[:])

            # pre-load the Exp act table during the head
            nc.scalar.activation(dummy_e[:, :], warm_t[:, 0:8], Exp)

            # ---- PE clock warm-up on memset data while the head DMAs run --
            with tc.tile_pool(name="pwarm", bufs=1, space="PSUM") as pw_pool:
                pw = pw_pool.tile([A, 256], f32)
                for wi in range(22):
                    nc.tensor.matmul(
                        pw[:, :], lhsT=ones_bf[:, :], rhs=warm_t[:, :],
                        start=(wi == 0), stop=(wi == 21))

            def emit_vproj(pair, vps_pool):
                vt = vps_pool.tile([128, 2, 2 * A], f32, name="vt", tag="vt")
                for j in range(2):
                    nc.tensor.matmul(
                        vt[:, j, :],
                        lhsT=hsv[:, pair * 2 + j, :],
                        rhs=wv_sb[:, :],
                        start=True, stop=True)
                dst = v_all[:, pair * 2:(pair + 1) * 2, :, 0:A]
                src = vt.rearrange("p c (a b) -> p c a b", a=2)
                if pair % 2 == 0:
                    nc.scalar.activation(dst, src, Copy)
                else:
                    nc.vector.tensor_copy(dst, src)

            # ---- v projection, first half: covers the weight DMA head ----
            with tc.tile_pool(name="vps1", bufs=4, space="PSUM") as vps1:
                for pair in range(NB // 4):
                    emit_vproj(pair, vps1)

            # ---- interleaved Wq/Wk projection streams ----
            with tc.tile_pool(name="wtq", bufs=2) as w_pool_q, \
                 tc.tile_pool(name="wtk", bufs=4) as w_pool_k, \
                 tc.tile_pool(name="stages", bufs=2) as st_pool, \
                 tc.tile_pool(name="pp", bufs=4, space="PSUM") as pp_pool:

                stage_q = st_pool.tile([128, BC, NH, 128], f16, name="sq",
                                       tag="st")
                stage_k = st_pool.tile([128, BC, NH, 128], f16, name="sk",
                                       tag="st")

                def issue_chunk(dma_eng, w_pool, w_d, tg):
                    wt = w_pool.tile([128, WCHUNK, KTILES, 128], f16,
                                     name="wt", tag="wt")
                    dma_eng.dma_start(
                        wt[:, :, :, :],
                        w_d[:, tg * WCHUNK:(tg + 1) * WCHUNK, :]
                        .rearrange("p t (kt c) -> p t kt c", c=128))
                    return wt

                def emit_group(wt, dest, stage, tg):
                    pp = pp_pool.tile([128, WCHUNK, ROWS], f32, name="pp",
                                      tag="pp")
                    for ti in range(WCHUNK):
                        for kt in range(KTILES):
                            nc.tensor.matmul(
                                pp[:, ti, :],
                                lhsT=wt[:, ti, kt, :],
                                rhs=hsT[:, kt, :],
                                start=(kt == 0),
                                stop=(kt == KTILES - 1))
                    src = pp.rearrange(
                        "p ti (b n sp) -> p (b n) ti sp", n=NH, sp=4)
                    t0 = tg * WCHUNK
                    dv = dest[:, 0, :, :, :].rearrange(
                        "p b n (f sp) -> p (b n) f sp", sp=4)
                    sv = stage[:, :, :, :].rearrange(
                        "p b n (f sp) -> p (b n) f sp", sp=4)
                    nc.scalar.activation(
                        dv[:, :, t0:t0 + WCHUNK, :], src[0:64], Copy)
                    nc.vector.tensor_copy(
                        sv[64:128, :, t0:t0 + WCHUNK, :], src[64:128])

                # deep prefetch for the wk stream (scalar queue also carries
                # the per-group jh0 evacs; distance-3 issues never block)
                wk_tiles = [issue_chunk(nc.scalar, w_pool_k, wk_d, t)
                            for t in range(3)]
                for tg in range(NCHUNK):
                    wq_t = issue_chunk(nc.sync, w_pool_q, wq_d, tg)
                    emit_group(wq_t, qt, stage_q, tg)
                    if tg + 3 < NCHUNK:
                        wk_tiles.append(
                            issue_chunk(nc.scalar, w_pool_k, wk_d, tg + 3))
                    emit_group(wk_tiles[tg], kt_, stage_k, tg)
                    if tg == NCHUNK - 1:
                        # qt partition shift 64..127 -> 0..63 (16KB runs)
                        nc.gpsimd.dma_start(
                            qt[:, 1, :, :, :], stage_q[64:128, :, :, :])

                # kt partition shift, split across two idle queues
                h = BC // 2
                nc.gpsimd.dma_start(
                    kt_[:, 1, 0:h, :, :], stage_k[64:128, 0:h, :, :])
                nc.sync.dma_start(
                    kt_[:, 1, h:BC, :, :], stage_k[64:128, h:BC, :, :])

                # ---- v projection, second half: covers the kt shift ----
                with tc.tile_pool(name="vps2", bufs=4, space="PSUM") as vps2:
                    for pair in range(NB // 4, NB // 2):
                        emit_vproj(pair, vps2)

            # ---- attention: transpose-free z, TRANSPOSED av (full-M ez
            # passes, denominator rides as rhs column 64), per-partition
            # reciprocal normalize, PE transpose back to the na-major ut
            # layout the residual expects.  Pipelined: exp[b+1] emitted
            # before av[b]; transpose runs one batch behind. ----
            with tc.tile_pool(name="zps", bufs=2, space="PSUM") as z_pool, \
                 tc.tile_pool(name="aps", bufs=2, space="PSUM") as a_pool, \
                 tc.tile_pool(name="tps", bufs=1, space="PSUM") as t_pool, \
                 tc.tile_pool(name="expz", bufs=3) as e_pool, \
                 tc.tile_pool(name="recs", bufs=2) as rec_pool, \
                 tc.tile_pool(name="utts", bufs=2) as utt_pool, \
                 tc.tile_pool(name="rps", bufs=1, space="PSUM") as r_pool, \
                 tc.tile_pool(name="fo", bufs=2) as f_pool:

                def emit_zt_exp(b):
                    zt = z_pool.tile([128, 4, 256], f32, name="zt", tag="zt")
                    for nh in range(NH):
                        for h in range(2):
                            nc.tensor.matmul(
                                zt[:, nh * 2 + h, :],
                                lhsT=kt_[:, h, b, nh, :],
                                rhs=qt[:, :, b, nh, :],
                                start=True, stop=True)
                    ez = e_pool.tile([128, 4, 256], bf16, name="ez", tag="ez")
                    nc.scalar.activation(
                        ez[:, :, :].rearrange("p a b -> p (a b)"),
                        zt[:, :, :].rearrange("p a b -> p (a b)"), Exp)
                    return ez

                def emit_avt(b, cur):
                    avT = a_pool.tile([128, 2, 2, A + 1], f32, name="avT",
                                      tag="avT")
                    for qh in range(2):
                        for nh in range(NH):
                            for kk in range(2):
                                nc.tensor.matmul(
                                    avT[:, qh, nh, :],
                                    lhsT=cur[:, nh * 2 + kk,
                                             qh * 128:(qh + 1) * 128],
                                    rhs=v_all[:, b * NH + nh, kk, :],
                                    start=(kk == 0), stop=(kk == 1))
                    rec = rec_pool.tile([128, 2, 2, 1], f32, name="rec",
                                        tag="rec")
                    nc.vector.reciprocal_approx_fast(
                        rec[:, :, :, :], avT[:, :, :, A:A + 1])
                    utT = utt_pool.tile([128, 2, 2, A], f16, name="utT",
                                        tag="utT")
                    for qh in range(2):
                        for nh in range(NH):
                            nc.vector.tensor_scalar_mul(
                                utT[:, qh, nh, :], avT[:, qh, nh, 0:A],
                                rec[:, qh, nh, :])
                    return utT

                def emit_tr(b, utT):
                    trp = t_pool.tile([128, 2, 128], f32, name="trp",
                                      tag="trp")
                    for qh in range(2):
                        nc.tensor.transpose(
                            trp[:, qh, :],
                            utT[:, qh, :, :].rearrange("p a b -> p (a b)"),
                            ident[:, :])
                    nc.gpsimd.tensor_copy(ut[:, b, :, :], trp[:, :, :])

                def emit_resid(bp):
                    rp = r_pool.tile([128, 2, 128], f32, name="rp", tag="rp")
                    for g in range(2):
                        for jh in range(2):
                            nc.tensor.matmul(
                                rp[g * 64:(g + 1) * 64, :, :],
                                lhsT=wres_sb[g * 64:(g + 1) * 64, jh, :],
                                rhs=ut[g * 64:(g + 1) * 64,
                                       bp * 2:(bp + 1) * 2, jh, :],
                                start=(jh == 0), stop=(jh == 1),
                                tile_position=(g * 64, g * 64))
                    fo = f_pool.tile([128, 2, 128], f16, name="fo", tag="fo")
                    nc.vector.tensor_scalar(
                        fo[:, :, :].rearrange("p a b -> p (a b)"),
                        rp[:, :, :].rearrange("p a b -> p (a b)"),
                        bias_sb[:, :], 0.0, Add, Max)
                    nc.sync.dma_start(
                        out_d[:, bp * 256:(bp + 1) * 256],
                        fo[:, :, :].rearrange("p a b -> p (a b)"))

                ezs = emit_zt_exp(0)
                prev_utT = None
                for b in range(BC):
                    cur = ezs
                    if b + 1 < BC:
                        ezs = emit_zt_exp(b + 1)
                    utT = emit_avt(b, cur)
                    if prev_utT is not None:
                        emit_tr(b - 1, prev_utT)
                    prev_utT = utT
                    if b >= 4 and b % 2 == 0:
                        emit_resid(b // 2 - 2)
                emit_tr(BC - 1, prev_utT)
                emit_resid(BC // 2 - 2)
                emit_resid(BC // 2 - 1)
    nc.compile()
    return nc


def _get_nc():
    global _NC_CACHE
    if _NC_CACHE is None:
        _NC_CACHE = build_bass()
    return _NC_CACHE


def _prep_weight(W):
    # (CD, ND) -> (128, TTILES, KTILES*128): [p, t, kt*128+j] = W[kt*128+p, t*128+j]
    return np.ascontiguousarray(
        W.astype(np.float16).reshape(KTILES, 128, TTILES, 128)
        .transpose(1, 2, 0, 3).reshape(128, TTILES, KTILES * 128))


def make_in_maps(Hs, Wq, Wk, Wv, Wres_w, Wres_b):
    wq16 = _prep_weight(Wq)
    wk16 = _prep_weight(Wk)
    wv16 = Wv.astype(np.float16)
    wres16 = Wres_w.astype(np.float16)
    bias = Wres_b.astype(np.float32).reshape(E, 1)
    hs16 = Hs.astype(np.float16)
    maps = []
    for c in range(NCORES):
        sh = hs16[c * BC:(c + 1) * BC]                      # (BC, S, CD)
        hs2d = sh.reshape(ROWS, CD)
        hst = np.ascontiguousarray(
            hs2d.reshape(ROWS, KTILES, 128).transpose(2, 1, 0))
        # v rows in sigma' order (f*4+sp):
        # hsv[pi*64+e, q, f*4+sp] = Hs[b, nh*4+sp, f, e]; bn = 2q+pi = b*NH+nh
        arr = sh.reshape(NB, 4, F, E).transpose(0, 2, 1, 3).reshape(NB, 128, E)
        hsv = np.ascontiguousarray(arr.transpose(2, 0, 1))  # (E, NB, sigma)
        maps.append({
            "hst": hst, "hsv": hsv,
            "wq": wq16, "wk": wk16, "wv": wv16, "wres": wres16, "bias": bias,
        })
    return maps


def _unpack_out(o):
    # o: (128, (BC//2)*256) rows (g, e), cols (bp, b2, f, sp) -> (BC, S, F*E)
    o = o.reshape(2, 64, BC // 2, 2, F, 4)
    return np.ascontiguousarray(
        o.transpose(2, 3, 0, 5, 4, 1)).reshape(BC, S, F * E)


def kernel(Hs, Wq, Wk, Wv, Wres_w, Wres_b):
    from concourse.bass_utils import run_bass_kernel_spmd
    nc = _get_nc()
    in_maps = make_in_maps(Hs, Wq, Wk, Wv, Wres_w, Wres_b)
    res = run_bass_kernel_spmd(nc, in_maps, list(range(NCORES)))
    out = np.concatenate(
        [_unpack_out(np.asarray(res.results[c]["out"]))
         for c in range(NCORES)], axis=0)
    return out.astype(np.float32)


if __name__ == "__main__":
    nc = build_bass()
    print("built OK; instructions:",
          sum(len(bb.instructions) for fn in nc.m.functions
              for bb in fn.blocks))
